# revision 1
# baseline (speedup 1.0000x reference)
"""DiffPool GNN kernel for one TRN2 chip (8 NeuronCores), Bass/Tile.

Math (reference):
    spmm(H) = segment_sum(edge_val[:,None] * H[edge_col], edge_row, N)
    S = softmax(relu(spmm(x @ W_pool)))         # [N, K]
    Z = relu(spmm(x @ W_embed))                 # [N, F]
    coarse_A = S.T @ spmm(S)                    # [K, K]
    coarse_X = S.T @ Z                          # [K, F]

Key reformulation: spmm(x @ W) == spmm(x) @ W, so a single SpMM Y = A@x
feeds both the pool and embed branches.  Only two SpMMs total (A@x, A@S).

Distribution: nodes are range-partitioned across the 8 cores by destination
row; each core owns the edges whose destination falls in its range.  The
segment sum runs on the TensorEngine: edges are grouped into 128-edge tiles
whose destinations all fall in one 128-row block, and for each tile a
val-weighted one-hot matrix M[pe, d] = val_pe * (d == dest_pe) multiplies
the gathered source rows G[pe, f], PSUM-accumulating per block.

Data movement strategy (Trainium's SWDGE descriptor generation costs ~8ns
per gathered row on the Q7, so per-edge indirect DMA is minimized):
  - SpMM-1 sources (x rows per edge) are pre-gathered BY THE HOST into a
    dense [128, T, 128] bf16 stream, so SpMM-1 is pure streaming DMA.
  - The M planes are pre-built BY THE HOST (dense bf16 stream, reused for
    both SpMMs since they share the edge ordering).
  - SpMM-2 sources are rows of S (computed on device), fetched with
    gpsimd.dma_gather from the all-gathered S in DRAM (int16 indices =>
    edges are pre-sorted into two source ranges split at 32768).
The K x K / K x F coarse outputs are PSUM-accumulated per block and
all-reduced at the end.
"""

import os
import sys
import types
import importlib.util
import numpy as np
import ml_dtypes

from concourse import bass, bacc, tile, mybir, library_config
from concourse.bass_utils import run_bass_kernel_spmd

BF16 = ml_dtypes.bfloat16
F32 = mybir.dt.float32
DBF = mybir.dt.bfloat16


def _install_profile_hook():
    """Register the axon NTFF profiling hook if the image's antenv lacks it."""
    try:
        import antenv.axon_hooks  # noqa: F401
        return
    except ImportError:
        pass
    try:
        spec = importlib.util.spec_from_file_location(
            "trn_boot", "/root/.axon_site/trn_agent_boot/trn_boot.py")
        trn_boot = importlib.util.module_from_spec(spec)
        spec.loader.exec_module(trn_boot)
        hook = trn_boot._ntff_profile_via_ctypes("/opt/axon/libaxon_pjrt.so")
        mod = types.ModuleType("antenv.axon_hooks")
        mod.get_axon_ntff_profile_hook = lambda: hook
        sys.modules["antenv.axon_hooks"] = mod
    except Exception:
        pass


class Cfg:
    def __init__(self, n_nodes, n_edges, cores, f_in, k_clust, grp_blocks,
                 split=32768):
        assert n_nodes % cores == 0
        self.N = n_nodes
        self.E = n_edges
        self.CORES = cores
        self.F = f_in          # feature dim == 128
        self.K = k_clust       # cluster dim == 128
        self.PN = n_nodes // cores
        self.BLK = 128
        self.NBLK = (self.PN + 127) // 128
        self.GRP = grp_blocks
        self.NGRP = (self.NBLK + grp_blocks - 1) // grp_blocks
        self.SPLIT = split


def _prep(cfg, x, edge_row, edge_col, edge_val):
    """Partition+sort+pad edges; build per-core Gx / M / idx planes."""
    c = cfg
    x_bf = np.ascontiguousarray(np.asarray(x, np.float32)).astype(BF16)
    owner = edge_row // c.PN
    nseg = c.NBLK * 2

    cores_sorted = []
    counts = np.zeros((c.CORES, nseg), np.int64)
    for m in range(c.CORES):
        sel = owner == m
        er = edge_row[sel] - m * c.PN
        ec = edge_col[sel]
        ev = edge_val[sel]
        blk = er // c.BLK
        rng = (ec >= c.SPLIT).astype(np.int64)
        seg = blk * 2 + rng
        order = np.argsort(seg, kind="stable")
        er, ec, ev = er[order], ec[order], ev[order]
        counts[m] = np.bincount(seg[order], minlength=nseg)
        cores_sorted.append((er, ec, ev, counts[m]))

    seg_tiles = (counts.max(axis=0) + 127) // 128

    seg_order = []
    for g in range(c.NGRP):
        bl = range(g * c.GRP, min(c.NBLK, (g + 1) * c.GRP))
        for r in (0, 1):
            for b in bl:
                seg_order.append(b * 2 + r)
    tiles_in_order = [int(seg_tiles[s]) for s in seg_order]
    T_total = int(sum(tiles_in_order))

    planes = []
    for m in range(c.CORES):
        er, ec, ev, cnt = cores_sorted[m]
        seg_start = np.zeros(nseg + 1, np.int64)
        seg_start[1:] = np.cumsum(cnt)
        cols = np.zeros(T_total * 128, np.int64)
        dest = np.zeros(T_total * 128, np.int64)
        val = np.zeros(T_total * 128, np.float32)
        idx16 = np.zeros(T_total * 128, np.int16)
        pos = 0
        for s, nt in zip(seg_order, tiles_in_order):
            b, r = s // 2, s % 2
            a0, a1 = seg_start[s], seg_start[s + 1]
            n = a1 - a0
            cols[pos:pos + n] = ec[a0:a1]
            cols[pos + n:pos + nt * 128] = r * c.SPLIT
            dest[pos:pos + n] = er[a0:a1] - b * c.BLK
            val[pos:pos + n] = ev[a0:a1]
            idx16[pos:pos + nt * 128] = (cols[pos:pos + nt * 128]
                                         - r * c.SPLIT).astype(np.int16)
            pos += nt * 128
        assert pos == T_total * 128

        # Gx plane: [128, T, 128] bf16, Gx[pe, t, :] = x[col of edge 128t+pe]
        gx = x_bf[cols].reshape(T_total, 128, c.F).transpose(1, 0, 2)
        gx = np.ascontiguousarray(gx)
        # M plane: [128, T, 128] bf16, M[pe, t, d] = val * (d == dest)
        mv = np.zeros((T_total * 128, 128), BF16)
        mv[np.arange(T_total * 128), dest] = val.astype(BF16)
        mv = np.ascontiguousarray(
            mv.reshape(T_total, 128, 128).transpose(1, 0, 2))
        idx_plane = np.tile(idx16.reshape(-1, 16).T, (8, 1)).copy()
        planes.append({"gx": gx, "mv": mv, "idx": idx_plane})

    return tiles_in_order, planes


def _build(cfg, tiles_in_order):
    c = cfg
    T_total = sum(tiles_in_order)

    call_info = []       # [(g, r, col0_tiles, [(b, off, nt), ...]), ...]
    pos = 0
    oi = 0
    for g in range(c.NGRP):
        bl = list(range(g * c.GRP, min(c.NBLK, (g + 1) * c.GRP)))
        for r in (0, 1):
            off = 0
            entries = []
            for b in bl:
                nt = tiles_in_order[oi]
                oi += 1
                entries.append((b, off, nt))
                off += nt
            call_info.append((g, r, pos, entries))
            pos += off
    GRPMAX = max(
        sum(nt for _, _, nt in call_info[2 * g][3])
        + sum(nt for _, _, nt in call_info[2 * g + 1][3])
        for g in range(c.NGRP))

    nc = bacc.Bacc("TRN2", target_bir_lowering=False, debug=False,
                   num_devices=c.CORES)
    gx_d = nc.dram_tensor("gx", [128, T_total, c.F], DBF,
                          kind="ExternalInput").ap()
    mv_d = nc.dram_tensor("mv", [128, T_total, 128], DBF,
                          kind="ExternalInput").ap()
    idx_d = nc.dram_tensor("idx", [128, T_total * 8], mybir.dt.int16,
                           kind="ExternalInput").ap()
    rmask_d = nc.dram_tensor("rmask", [128, 1], F32, kind="ExternalInput").ap()
    wp_d = nc.dram_tensor("wp", [c.F, c.K], DBF, kind="ExternalInput").ap()
    we_d = nc.dram_tensor("we", [c.F, c.K], DBF, kind="ExternalInput").ap()
    ca_d = nc.dram_tensor("coarse_A", [c.K, c.K], F32, kind="ExternalOutput").ap()
    cx_d = nc.dram_tensor("coarse_X", [c.K, c.F], F32, kind="ExternalOutput").ap()

    s_bounce = nc.dram_tensor("s_bounce", [c.PN, c.K], DBF).ap()
    s_full = nc.dram_tensor("s_full", [c.N, c.K], DBF).ap()
    cc_in = nc.dram_tensor("cc_in", [128, 256], F32).ap()
    cc_out = nc.dram_tensor("cc_out", [128, 256], F32).ap()

    last_rows = c.PN - (c.NBLK - 1) * c.BLK

    with tile.TileContext(nc) as tc:
        with (
            tc.tile_pool(name="const", bufs=1) as constp,
            tc.tile_pool(name="gbuf", bufs=3) as gpool,
            tc.tile_pool(name="mbuf", bufs=3) as mpool,
            tc.tile_pool(name="node", bufs=1) as nodep,
            tc.tile_pool(name="small", bufs=4) as smallp,
            tc.tile_pool(name="py", bufs=4, space="PSUM") as psum_y,
            tc.tile_pool(name="pde", bufs=2, space="PSUM") as psum_de,
            tc.tile_pool(name="pca", bufs=1, space="PSUM") as psum_ca,
            tc.tile_pool(name="pcx", bufs=1, space="PSUM") as psum_cx,
        ):
            nc.gpsimd.load_library(library_config.mlp)

            idx_sb = constp.tile([128, T_total * 8], mybir.dt.int16)
            rmask_sb = constp.tile([128, 1], F32)
            wp_sb = constp.tile([c.F, c.K], DBF)
            we_sb = constp.tile([c.F, c.K], DBF)
            nc.sync.dma_start(out=idx_sb[:, :], in_=idx_d[:, :])
            nc.sync.dma_start(out=rmask_sb[:, :], in_=rmask_d[:, :])
            nc.sync.dma_start(out=wp_sb[:, :], in_=wp_d[:, :])
            nc.sync.dma_start(out=we_sb[:, :], in_=we_d[:, :])

            yt_sb = nodep.tile([128, c.NBLK, 128], DBF)
            s_sb = nodep.tile([128, c.NBLK, c.K], DBF)
            z_sb = nodep.tile([128, c.NBLK, c.K], DBF)

            p_ca = psum_ca.tile([128, c.K], F32)
            p_cx = psum_cx.tile([128, c.K], F32)

            def spmm(phase):
                """phase 1: psum[f,d] = Gx^T M (Y^T), dense-streamed Gx;
                   phase 2: psum[d,k] = M^T Gs, Gs gathered from s_full."""
                for gi in range(c.NGRP):
                    _, _, g0, entries0 = call_info[gi * 2]
                    _, _, g1, entries1 = call_info[gi * 2 + 1]
                    t0 = sum(nt for _, _, nt in entries0)
                    t1 = sum(nt for _, _, nt in entries1)
                    tg = t0 + t1
                    if tg == 0:
                        continue
                    mb = mpool.tile([128, GRPMAX, 128], DBF, tag="mb")
                    nc.scalar.dma_start(out=mb[:, 0:tg, :],
                                        in_=mv_d[:, g0:g0 + tg, :])
                    gb = gpool.tile([128, GRPMAX, 128], DBF, tag="gb")
                    if phase == 1:
                        nc.sync.dma_start(out=gb[:, 0:tg, :],
                                          in_=gx_d[:, g0:g0 + tg, :])
                    else:
                        if t0:
                            nc.gpsimd.dma_gather(
                                out_ap=gb[:, 0:t0, :],
                                in_ap=s_full[0:min(c.SPLIT, c.N), :],
                                idxs_ap=idx_sb[:, g0 * 8:(g0 + t0) * 8],
                                num_idxs=t0 * 128, num_idxs_reg=t0 * 128,
                                elem_size=c.F, single_packet=False)
                        if t1:
                            nc.gpsimd.dma_gather(
                                out_ap=gb[:, t0:tg, :],
                                in_ap=s_full[c.SPLIT:c.N, :],
                                idxs_ap=idx_sb[:, g1 * 8:(g1 + t1) * 8],
                                num_idxs=t1 * 128, num_idxs_reg=t1 * 128,
                                elem_size=c.F, single_packet=False)

                    bl = list(range(gi * c.GRP, min(c.NBLK, (gi + 1) * c.GRP)))
                    for b in bl:
                        chunks = []
                        for (bb, off, nt) in entries0:
                            if bb == b and nt > 0:
                                chunks.append((off, nt))
                        for (bb, off, nt) in entries1:
                            if bb == b and nt > 0:
                                chunks.append((t0 + off, nt))
                        total = sum(nt for _, nt in chunks)
                        if total == 0:
                            continue
                        acc = psum_y.tile([128, 128], F32, tag="py")
                        done = 0
                        for (off, nt) in chunks:
                            for j in range(nt):
                                first = done == 0
                                done += 1
                                last = done == total
                                if phase == 1:
                                    lhsT, rhs = gb[:, off + j, :], mb[:, off + j, :]
                                else:
                                    lhsT, rhs = mb[:, off + j, :], gb[:, off + j, :]
                                nc.tensor.matmul(acc[:, :], lhsT, rhs,
                                                 start=first, stop=last)
                        yield b, acc

            # ---- SpMM-1 + dense + softmax --------------------------------
            for b, acc in spmm(1):
                nc.vector.tensor_copy(yt_sb[:, b, :], acc[:, :])
                pde = psum_de.tile([128, 2, 128], F32, tag="pde")
                nc.tensor.matmul(pde[:, 0, :], yt_sb[:, b, :], wp_sb[:, :])
                nc.tensor.matmul(pde[:, 1, :], yt_sb[:, b, :], we_sb[:, :])
                lg = smallp.tile([128, 128], F32, tag="lg")
                mx = smallp.tile([128, 1], F32, tag="mx")
                ex = smallp.tile([128, 128], F32, tag="ex")
                sm = smallp.tile([128, 1], F32, tag="sm")
                rc = smallp.tile([128, 1], F32, tag="rc")
                nc.vector.tensor_scalar_max(lg[:, :], pde[:, 0, :], 0.0)
                nc.vector.tensor_reduce(mx[:, :], lg[:, :],
                                        axis=mybir.AxisListType.X,
                                        op=mybir.AluOpType.max, negate=True)
                nc.scalar.activation(ex[:, :], lg[:, :],
                                     mybir.ActivationFunctionType.Exp,
                                     bias=mx[:, 0:1], scale=1.0,
                                     accum_out=sm[:, 0:1])
                nc.vector.reciprocal(rc[:, :], sm[:, :])
                if b == c.NBLK - 1 and last_rows < 128:
                    nc.vector.tensor_scalar(
                        out=s_sb[:, b, :], in0=ex[:, :], scalar1=rc[:, 0:1],
                        scalar2=rmask_sb[:, 0:1], op0=mybir.AluOpType.mult,
                        op1=mybir.AluOpType.mult)
                    nc.vector.tensor_scalar(
                        out=z_sb[:, b, :], in0=pde[:, 1, :], scalar1=0.0,
                        scalar2=rmask_sb[:, 0:1], op0=mybir.AluOpType.max,
                        op1=mybir.AluOpType.mult)
                else:
                    nc.vector.tensor_scalar_mul(s_sb[:, b, :], ex[:, :],
                                                rc[:, 0:1])
                    nc.vector.tensor_scalar_max(z_sb[:, b, :], pde[:, 1, :], 0.0)
                nc.tensor.matmul(p_cx[:, :], s_sb[:, b, :], z_sb[:, b, :],
                                 start=(b == 0), stop=(b == c.NBLK - 1))

            # ---- AllGather S ---------------------------------------------
            nfull = c.NBLK - 1
            nc.sync.dma_start(
                out=s_bounce[0:nfull * 128, :].rearrange("(b p) k -> p b k", p=128),
                in_=s_sb[:, 0:nfull, :])
            nc.sync.dma_start(
                out=s_bounce[nfull * 128:c.PN, :],
                in_=s_sb[0:last_rows, nfull, :])
            nc.gpsimd.collective_compute(
                "AllGather", mybir.AluOpType.bypass,
                replica_groups=[list(range(c.CORES))],
                ins=[s_bounce.opt()], outs=[s_full.opt()])

            # ---- SpMM-2 (A @ S) + coarse_A -------------------------------
            for b, acc in spmm(2):
                asb = smallp.tile([128, 128], DBF, tag="asb")
                nc.vector.tensor_copy(asb[:, :], acc[:, :])
                nc.tensor.matmul(p_ca[:, :], s_sb[:, b, :], asb[:, :],
                                 start=(b == 0), stop=(b == c.NBLK - 1))

            # ---- AllReduce + outputs -------------------------------------
            cc_sb = smallp.tile([128, 256], F32, tag="cc")
            nc.vector.tensor_copy(cc_sb[:, 0:128], p_ca[:, :])
            nc.vector.tensor_copy(cc_sb[:, 128:256], p_cx[:, :])
            nc.sync.dma_start(out=cc_in[:, :], in_=cc_sb[:, :])
            nc.gpsimd.collective_compute(
                "AllReduce", mybir.AluOpType.add,
                replica_groups=[list(range(c.CORES))],
                ins=[cc_in.opt()], outs=[cc_out.opt()])
            out_sb = smallp.tile([128, 256], F32, tag="cc")
            nc.sync.dma_start(out=out_sb[:, :], in_=cc_out[:, :])
            nc.sync.dma_start(out=ca_d[:, :], in_=out_sb[:, 0:128])
            nc.sync.dma_start(out=cx_d[:, :], in_=out_sb[:, 128:256])

    nc.compile()
    return nc


def _run(cfg, nc, planes, W_pool, W_embed, trace=False):
    c = cfg
    rmask = np.zeros((128, 1), np.float32)
    lr = c.PN - (c.NBLK - 1) * 128 if c.PN % 128 else 128
    rmask[:lr] = 1.0
    wp = np.ascontiguousarray(np.asarray(W_pool, np.float32)).astype(BF16)
    we = np.ascontiguousarray(np.asarray(W_embed, np.float32)).astype(BF16)
    in_maps = []
    for m in range(c.CORES):
        in_maps.append({
            "rmask": rmask, "wp": wp, "we": we,
            "gx": planes[m]["gx"], "mv": planes[m]["mv"],
            "idx": planes[m]["idx"],
        })
    res = run_bass_kernel_spmd(nc, in_maps, list(range(c.CORES)), trace=trace)
    ca = np.asarray(res.results[0]["coarse_A"], np.float32)
    cx = np.asarray(res.results[0]["coarse_X"], np.float32)
    return ca, cx, res


FULL = Cfg(n_nodes=50000, n_edges=1600000, cores=8, f_in=128, k_clust=128,
           grp_blocks=2)


def kernel(x, edge_row, edge_col, edge_val, W_pool, W_embed):
    _install_profile_hook()
    x = np.asarray(x, np.float32)
    edge_row = np.asarray(edge_row, np.int32)
    edge_col = np.asarray(edge_col, np.int32)
    edge_val = np.asarray(edge_val, np.float32)

    tiles_in_order, planes = _prep(FULL, x, edge_row, edge_col, edge_val)
    nc = _build(FULL, tiles_in_order)
    ca, cx, _ = _run(FULL, nc, planes, W_pool, W_embed)
    return ca, cx



# revision 5
# speedup vs baseline: 1.5143x; 1.5143x over previous
"""DiffPool GNN kernel for one TRN2 chip (8 NeuronCores), Bass/Tile.

Math (reference):
    spmm(H) = segment_sum(edge_val[:,None] * H[edge_col], edge_row, N)
    S = softmax(relu(spmm(x @ W_pool)))         # [N, K]
    Z = relu(spmm(x @ W_embed))                 # [N, F]
    coarse_A = S.T @ spmm(S)                    # [K, K]
    coarse_X = S.T @ Z                          # [K, F]

Key reformulation: spmm(x @ W) == spmm(x) @ W, so a single SpMM Y = A@x
feeds both the pool and embed branches.  Only two SpMMs total (A@x, A@S).

Distribution: nodes are range-partitioned across the 8 cores by destination
row; each core owns the edges whose destination falls in its range.

Phase 1 (Y = A@x): edges grouped into 128-edge tiles per 128-row dest
block; host pre-gathers source rows into a dense bf16 stream Gx and builds
val-weighted one-hot planes M; TensorEngine computes Y per block as
Gx^T @ M with PSUM accumulation.  Pure streaming DMA, no gathers.

Phase 2 (A@S then S^T(A@S)): S is runtime data, so its per-edge gather
would cost ~10ns/row on the SWDGE descriptor path (measured; the serial
descriptor stage dominates at ~2ms for all edges).  Hybrid split by source
column c* balances two engines:
  - cols < c*: descriptor-path dma_gather from the all-gathered S (runs on
    the gpsimd/SWDGE ring), consumed by one-hot M2 matmuls (bf16).
  - cols >= c*: DENSE fp8 block-SpMM: host builds dense 128x128 A^T blocks
    (fp8e4, ~10.7 edges each); S blocks live SBUF-resident in fp8;
    DoubleRow matmuls contract 256 rows per instruction.  This path costs
    DMA bytes + PE time instead of serial descriptors.
The K x K / K x F coarse outputs are PSUM-accumulated per block and
all-reduced at the end.
"""

import os
import sys
import types
import importlib.util
import numpy as np
import ml_dtypes

from concourse import bass, bacc, tile, mybir, library_config
from concourse.bass_utils import run_bass_kernel_spmd

BF16 = ml_dtypes.bfloat16
F8 = ml_dtypes.float8_e4m3fn
F32 = mybir.dt.float32
DBF = mybir.dt.bfloat16
DF8 = mybir.dt.float8e4


def _install_profile_hook():
    """Register the axon NTFF profiling hook if the image's antenv lacks it."""
    try:
        import antenv.axon_hooks  # noqa: F401
        return
    except ImportError:
        pass
    try:
        spec = importlib.util.spec_from_file_location(
            "trn_boot", "/root/.axon_site/trn_agent_boot/trn_boot.py")
        trn_boot = importlib.util.module_from_spec(spec)
        spec.loader.exec_module(trn_boot)
        hook = trn_boot._ntff_profile_via_ctypes("/opt/axon/libaxon_pjrt.so")
        mod = types.ModuleType("antenv.axon_hooks")
        mod.get_axon_ntff_profile_hook = lambda: hook
        sys.modules["antenv.axon_hooks"] = mod
    except Exception:
        pass


class Cfg:
    def __init__(self, n_nodes, n_edges, cores, f_in, k_clust, grp_blocks,
                 cstar_blocks=None, split=None):
        assert n_nodes % cores == 0
        self.N = n_nodes
        self.E = n_edges
        self.CORES = cores
        self.F = f_in          # feature dim == 128
        self.K = k_clust       # cluster dim == 128
        self.PN = n_nodes // cores
        self.BLK = 128
        self.NBLK = (self.PN + 127) // 128
        self.GRP = grp_blocks
        self.NGRP = (self.NBLK + grp_blocks - 1) // grp_blocks
        # source-col blocks: total (ceil), split point for hybrid phase 2
        self.WALL = (n_nodes + 127) // 128
        if cstar_blocks is None:
            cstar_blocks = self.WALL  # all gather, no dense
        self.WSTAR = cstar_blocks
        self.CSTAR = cstar_blocks * 128
        assert self.CSTAR <= 32768  # int16 gather indices
        # dense w-blocks padded to even count for DoubleRow pairing
        self.WDN = self.WALL - self.WSTAR
        if self.WDN % 2:
            self.WDN += 1


def _sort_by_dest(cfg, edge_row, edge_col, edge_val, sel):
    """Per-core edge lists sorted by dest block; per-seg counts."""
    c = cfg
    owner = edge_row // c.PN
    cores_sorted = []
    counts = np.zeros((c.CORES, c.NBLK), np.int64)
    for m in range(c.CORES):
        s = sel & (owner == m)
        er = edge_row[s] - m * c.PN
        ec = edge_col[s]
        ev = edge_val[s]
        seg = er // c.BLK
        order = np.argsort(seg, kind="stable")
        er, ec, ev = er[order], ec[order], ev[order]
        counts[m] = np.bincount(seg[order], minlength=c.NBLK)
        cores_sorted.append((er, ec, ev))
    return cores_sorted, counts


def _tile_stream(cfg, cores_sorted, counts, want_gx, x_bf):
    """Build per-core [128,T,128] Gx/M planes + int16 idx from edge lists."""
    c = cfg
    seg_tiles = (counts.max(axis=0) + 127) // 128
    tiles_in_order = [int(t) for t in seg_tiles]
    T_total = int(sum(tiles_in_order))
    planes = []
    for m in range(c.CORES):
        er, ec, ev = cores_sorted[m]
        cnt = counts[m]
        seg_start = np.zeros(c.NBLK + 1, np.int64)
        seg_start[1:] = np.cumsum(cnt)
        cols = np.zeros(T_total * 128, np.int64)
        dest = np.zeros(T_total * 128, np.int64)
        val = np.zeros(T_total * 128, np.float32)
        pos = 0
        for b, nt in enumerate(tiles_in_order):
            a0, a1 = seg_start[b], seg_start[b + 1]
            n = a1 - a0
            cols[pos:pos + n] = ec[a0:a1]
            dest[pos:pos + n] = er[a0:a1] - b * c.BLK
            val[pos:pos + n] = ev[a0:a1]
            pos += nt * 128
        assert pos == T_total * 128
        p = {}
        if want_gx:
            gx = x_bf[cols].reshape(T_total, 128, c.F).transpose(1, 0, 2)
            p["gx"] = np.ascontiguousarray(gx)
        mv = np.zeros((T_total * 128, 128), BF16)
        mv[np.arange(T_total * 128), dest] = val.astype(BF16)
        p["mv"] = np.ascontiguousarray(
            mv.reshape(T_total, 128, 128).transpose(1, 0, 2))
        idx16 = cols.astype(np.int16)
        p["idx"] = np.tile(idx16.reshape(-1, 16).T, (8, 1)).copy()
        planes.append(p)
    return tiles_in_order, planes


def _prep(cfg, x, edge_row, edge_col, edge_val):
    c = cfg
    x_bf = np.ascontiguousarray(np.asarray(x, np.float32)).astype(BF16)
    allsel = np.ones(edge_row.shape, bool)

    # phase 1: all edges, streamed Gx + M
    cs1, cnt1 = _sort_by_dest(c, edge_row, edge_col, edge_val, allsel)
    tiles1, planes1 = _tile_stream(c, cs1, cnt1, True, x_bf)

    # phase 2 gather path: cols < CSTAR
    gsel = edge_col < c.CSTAR
    cs2, cnt2 = _sort_by_dest(c, edge_row, edge_col, edge_val, gsel)
    tiles2, planes2 = _tile_stream(c, cs2, cnt2, False, None)

    # phase 2 dense path: cols >= CSTAR -> fp8 A^T blocks
    # layout per core: [NBLK, 128 j, WDN*128 d]; A8[b, j, (w-WSTAR)*128+d]
    planesA = []
    if c.WDN:
        owner = edge_row // c.PN
        for m in range(c.CORES):
            s = (~gsel) & (owner == m)
            er = (edge_row[s] - m * c.PN).astype(np.int64)
            ec = edge_col[s].astype(np.int64)
            ev = edge_val[s].astype(np.float32)
            b = er // 128
            d = er % 128
            w = ec // 128 - c.WSTAR
            j = ec % 128
            a = np.zeros(c.NBLK * 128 * c.WDN * 128, np.float32)
            flat = ((b * 128 + j) * c.WDN + w) * 128 + d
            np.add.at(a, flat, ev)
            planesA.append(
                a.reshape(c.NBLK, 128, c.WDN * 128).astype(F8))
    else:
        planesA = [np.zeros((c.NBLK, 128, 0), F8) for _ in range(c.CORES)]

    planes = []
    for m in range(c.CORES):
        p = {"gx": planes1[m]["gx"], "mv": planes1[m]["mv"],
             "mv2": planes2[m]["mv"], "idx2": planes2[m]["idx"],
             "ab": planesA[m]}
        planes.append(p)
    return tiles1, tiles2, planes


def _mk_groups(cfg, tiles_in_order):
    call_info = []
    pos = 0
    for g in range(cfg.NGRP):
        bl = range(g * cfg.GRP, min(cfg.NBLK, (g + 1) * cfg.GRP))
        off = 0
        entries = []
        for b in bl:
            nt = tiles_in_order[b]
            entries.append((b, off, nt))
            off += nt
        call_info.append((pos, entries))
        pos += off
    grpmax = max(sum(nt for _, _, nt in e) for _, e in call_info)
    return call_info, max(grpmax, 1)


def _build(cfg, tiles1, tiles2):
    c = cfg
    T1 = sum(tiles1)
    T2 = max(sum(tiles2), 1)
    ci1, GM1 = _mk_groups(c, tiles1)
    ci2, GM2 = _mk_groups(c, tiles2)
    GM = max(GM1, GM2)

    nc = bacc.Bacc("TRN2", target_bir_lowering=False, debug=False,
                   num_devices=c.CORES)
    gx_d = nc.dram_tensor("gx", [128, T1, c.F], DBF, kind="ExternalInput").ap()
    mv_d = nc.dram_tensor("mv", [128, T1, 128], DBF, kind="ExternalInput").ap()
    mv2_d = nc.dram_tensor("mv2", [128, T2, 128], DBF,
                           kind="ExternalInput").ap()
    idx2_d = nc.dram_tensor("idx2", [128, T2 * 8], mybir.dt.int16,
                            kind="ExternalInput").ap()
    ab_d = nc.dram_tensor("ab", [c.NBLK, 128, max(c.WDN, 1) * 128], DF8,
                          kind="ExternalInput").ap()
    rmask_d = nc.dram_tensor("rmask", [128, 1], F32, kind="ExternalInput").ap()
    wp_d = nc.dram_tensor("wp", [c.F, c.K], DBF, kind="ExternalInput").ap()
    we_d = nc.dram_tensor("we", [c.F, c.K], DBF, kind="ExternalInput").ap()
    ca_d = nc.dram_tensor("coarse_A", [c.K, c.K], F32,
                          kind="ExternalOutput").ap()
    cx_d = nc.dram_tensor("coarse_X", [c.K, c.F], F32,
                          kind="ExternalOutput").ap()

    s_bounce = nc.dram_tensor("s_bounce", [c.PN, c.K], DBF).ap()
    s_full = nc.dram_tensor("s_full", [c.N, c.K], DBF).ap()
    cc_in = nc.dram_tensor("cc_in", [128, 256], F32).ap()
    cc_out = nc.dram_tensor("cc_out", [128, 256], F32).ap()

    last_rows = c.PN - (c.NBLK - 1) * c.BLK

    with tile.TileContext(nc) as tc:
        with (
            tc.tile_pool(name="const", bufs=1) as constp,
            tc.tile_pool(name="gbuf", bufs=2) as gpool,
            tc.tile_pool(name="mbuf", bufs=2) as mpool,
            tc.tile_pool(name="node", bufs=1) as nodep,
            tc.tile_pool(name="small", bufs=4) as smallp,
            tc.tile_pool(name="abuf", bufs=3) as apool,
            tc.tile_pool(name="s8p", bufs=1) as s8pool,
            tc.tile_pool(name="py", bufs=4, space="PSUM") as psum_y,
            tc.tile_pool(name="pde", bufs=2, space="PSUM") as psum_de,
            tc.tile_pool(name="pca", bufs=1, space="PSUM") as psum_ca,
            tc.tile_pool(name="pcx", bufs=1, space="PSUM") as psum_cx,
        ):
            nc.gpsimd.load_library(library_config.mlp)

            idx_sb = constp.tile([128, T2 * 8], mybir.dt.int16)
            rmask_sb = constp.tile([128, 1], F32)
            wp_sb = constp.tile([c.F, c.K], DBF)
            we_sb = constp.tile([c.F, c.K], DBF)
            nc.sync.dma_start(out=idx_sb[:, :], in_=idx2_d[:, :])
            nc.sync.dma_start(out=rmask_sb[:, :], in_=rmask_d[:, :])
            nc.sync.dma_start(out=wp_sb[:, :], in_=wp_d[:, :])
            nc.sync.dma_start(out=we_sb[:, :], in_=we_d[:, :])

            yt_sb = nodep.tile([128, c.NBLK, 128], DBF)
            s_sb = nodep.tile([128, c.NBLK, c.K], DBF)
            z_sb = nodep.tile([128, c.NBLK, c.K], DBF)

            p_ca = psum_ca.tile([128, c.K], F32)
            p_cx = psum_cx.tile([128, c.K], F32)

            # ---- SpMM-1 + dense + softmax --------------------------------
            for gi in range(c.NGRP):
                g0, entries = ci1[gi]
                tg = sum(nt for _, _, nt in entries)
                if tg == 0:
                    continue
                mb = mpool.tile([128, GM, 128], DBF, tag="mb")
                nc.scalar.dma_start(out=mb[:, 0:tg, :],
                                    in_=mv_d[:, g0:g0 + tg, :])
                gb = gpool.tile([128, GM, 128], DBF, tag="gb")
                nc.sync.dma_start(out=gb[:, 0:tg, :],
                                  in_=gx_d[:, g0:g0 + tg, :])
                for (b, off, nt) in entries:
                    if nt == 0:
                        continue
                    acc = psum_y.tile([128, 128], F32, tag="py")
                    for t in range(nt):
                        nc.tensor.matmul(acc[:, :], gb[:, off + t, :],
                                         mb[:, off + t, :],
                                         start=(t == 0), stop=(t == nt - 1))
                    nc.vector.tensor_copy(yt_sb[:, b, :], acc[:, :])
                    pde = psum_de.tile([128, 2, 128], F32, tag="pde")
                    nc.tensor.matmul(pde[:, 0, :], yt_sb[:, b, :], wp_sb[:, :])
                    nc.tensor.matmul(pde[:, 1, :], yt_sb[:, b, :], we_sb[:, :])
                    lg = smallp.tile([128, 128], F32, tag="lg")
                    mx = smallp.tile([128, 1], F32, tag="mx")
                    ex = smallp.tile([128, 128], F32, tag="ex")
                    sm = smallp.tile([128, 1], F32, tag="sm")
                    rc = smallp.tile([128, 1], F32, tag="rc")
                    nc.vector.tensor_scalar_max(lg[:, :], pde[:, 0, :], 0.0)
                    nc.vector.tensor_reduce(mx[:, :], lg[:, :],
                                            axis=mybir.AxisListType.X,
                                            op=mybir.AluOpType.max,
                                            negate=True)
                    nc.scalar.activation(ex[:, :], lg[:, :],
                                         mybir.ActivationFunctionType.Exp,
                                         bias=mx[:, 0:1], scale=1.0,
                                         accum_out=sm[:, 0:1])
                    nc.vector.reciprocal(rc[:, :], sm[:, :])
                    if b == c.NBLK - 1 and last_rows < 128:
                        nc.vector.tensor_scalar(
                            out=s_sb[:, b, :], in0=ex[:, :],
                            scalar1=rc[:, 0:1], scalar2=rmask_sb[:, 0:1],
                            op0=mybir.AluOpType.mult,
                            op1=mybir.AluOpType.mult)
                        nc.vector.tensor_scalar(
                            out=z_sb[:, b, :], in0=pde[:, 1, :], scalar1=0.0,
                            scalar2=rmask_sb[:, 0:1],
                            op0=mybir.AluOpType.max,
                            op1=mybir.AluOpType.mult)
                    else:
                        nc.vector.tensor_scalar_mul(s_sb[:, b, :], ex[:, :],
                                                    rc[:, 0:1])
                        nc.vector.tensor_scalar_max(z_sb[:, b, :],
                                                    pde[:, 1, :], 0.0)
                    nc.tensor.matmul(p_cx[:, :], s_sb[:, b, :], z_sb[:, b, :],
                                     start=(b == 0), stop=(b == c.NBLK - 1))

            # ---- AllGather S ---------------------------------------------
            nfull = c.NBLK - 1
            nc.sync.dma_start(
                out=s_bounce[0:nfull * 128, :].rearrange(
                    "(b p) k -> p b k", p=128),
                in_=s_sb[:, 0:nfull, :])
            nc.sync.dma_start(
                out=s_bounce[nfull * 128:c.PN, :],
                in_=s_sb[0:last_rows, nfull, :])
            nc.gpsimd.collective_compute(
                "AllGather", mybir.AluOpType.bypass,
                replica_groups=[list(range(c.CORES))],
                ins=[s_bounce.opt()], outs=[s_full.opt()])

            # ---- S blocks >= CSTAR resident in SBUF as fp8 ---------------
            if c.WDN:
                s8 = s8pool.tile([128, c.WDN, 128], DF8)
                nc.vector.memset(s8[:, :, :], 0.0)
                CH = 8
                for w0 in range(0, c.WDN, CH):
                    wn = min(CH, c.WDN - w0)
                    r0 = (c.WSTAR + w0) * 128
                    rn = min(wn * 128, c.N - r0)
                    if rn <= 0:
                        break
                    tmp = smallp.tile([128, CH, 128], DBF, tag="scast")
                    full = rn // 128
                    if full:
                        nc.sync.dma_start(
                            out=tmp[:, 0:full, :],
                            in_=s_full[r0:r0 + full * 128, :].rearrange(
                                "(a p) k -> p a k", p=128))
                    rem = rn - full * 128
                    if rem > 0:
                        nc.vector.memset(tmp[:, full, :], 0.0)
                        nc.sync.dma_start(
                            out=tmp[0:rem, full, :],
                            in_=s_full[r0 + full * 128:r0 + rn, :])
                    nw = full + (1 if rem else 0)
                    nc.vector.tensor_copy(s8[:, w0:w0 + nw, :],
                                          tmp[:, 0:nw, :])

            # ---- SpMM-2 (A @ S): dense fp8 + gathered bf16 ---------------
            for gi in range(c.NGRP):
                g0, entries = ci2[gi]
                tg = sum(nt for _, _, nt in entries)
                mb = mpool.tile([128, GM, 128], DBF, tag="mb")
                if tg:
                    nc.scalar.dma_start(out=mb[:, 0:tg, :],
                                        in_=mv2_d[:, g0:g0 + tg, :])
                gb = gpool.tile([128, GM, 128], DBF, tag="gb")
                if tg:
                    nc.gpsimd.dma_gather(
                        out_ap=gb[:, 0:tg, :],
                        in_ap=s_full[0:c.CSTAR, :],
                        idxs_ap=idx_sb[:, g0 * 8:(g0 + tg) * 8],
                        num_idxs=tg * 128, num_idxs_reg=tg * 128,
                        elem_size=c.F, single_packet=False)
                for (b, off, nt) in entries:
                    acc = psum_y.tile([128, 128], F32, tag="py")
                    total = (c.WDN // 2 if c.WDN else 0) + nt
                    if total == 0:
                        continue
                    done = 0
                    ACH = 32
                    if c.WDN:
                        for w0 in range(0, c.WDN, ACH):
                            wn = min(ACH, c.WDN - w0)
                            abt = apool.tile([128, ACH, 128], DF8, tag="ab")
                            nc.sync.dma_start(
                                out=abt[:, 0:wn, :],
                                in_=ab_d[b, :, w0 * 128:(w0 + wn) * 128]
                                .rearrange("p (w d) -> p w d", d=128))
                            for wp in range(0, wn, 2):
                                done += 1
                                nc.tensor.matmul(
                                    acc[:, :], abt[:, wp:wp + 2, :],
                                    s8[:, w0 + wp:w0 + wp + 2, :],
                                    start=(done == 1), stop=(done == total),
                                    perf_mode=mybir.MatmulPerfMode.DoubleRow)
                    for t in range(nt):
                        done += 1
                        nc.tensor.matmul(acc[:, :], mb[:, off + t, :],
                                         gb[:, off + t, :],
                                         start=(done == 1),
                                         stop=(done == total))
                    asb = smallp.tile([128, 128], DBF, tag="asb")
                    nc.vector.tensor_copy(asb[:, :], acc[:, :])
                    nc.tensor.matmul(p_ca[:, :], s_sb[:, b, :], asb[:, :],
                                     start=(b == 0), stop=(b == c.NBLK - 1))

            # ---- AllReduce + outputs -------------------------------------
            cc_sb = smallp.tile([128, 256], F32, tag="cc")
            nc.vector.tensor_copy(cc_sb[:, 0:128], p_ca[:, :])
            nc.vector.tensor_copy(cc_sb[:, 128:256], p_cx[:, :])
            nc.sync.dma_start(out=cc_in[:, :], in_=cc_sb[:, :])
            nc.gpsimd.collective_compute(
                "AllReduce", mybir.AluOpType.add,
                replica_groups=[list(range(c.CORES))],
                ins=[cc_in.opt()], outs=[cc_out.opt()])
            out_sb = smallp.tile([128, 256], F32, tag="cc")
            nc.sync.dma_start(out=out_sb[:, :], in_=cc_out[:, :])
            nc.sync.dma_start(out=ca_d[:, :], in_=out_sb[:, 0:128])
            nc.sync.dma_start(out=cx_d[:, :], in_=out_sb[:, 128:256])

    nc.compile()
    return nc


def _run(cfg, nc, planes, W_pool, W_embed, trace=False):
    c = cfg
    rmask = np.zeros((128, 1), np.float32)
    lr = c.PN - (c.NBLK - 1) * 128 if c.PN % 128 else 128
    rmask[:lr] = 1.0
    wp = np.ascontiguousarray(np.asarray(W_pool, np.float32)).astype(BF16)
    we = np.ascontiguousarray(np.asarray(W_embed, np.float32)).astype(BF16)
    in_maps = []
    for m in range(c.CORES):
        in_maps.append({
            "rmask": rmask, "wp": wp, "we": we,
            "gx": planes[m]["gx"], "mv": planes[m]["mv"],
            "mv2": planes[m]["mv2"], "idx2": planes[m]["idx2"],
            "ab": planes[m]["ab"],
        })
    res = run_bass_kernel_spmd(nc, in_maps, list(range(c.CORES)), trace=trace)
    ca = np.asarray(res.results[0]["coarse_A"], np.float32)
    cx = np.asarray(res.results[0]["coarse_X"], np.float32)
    return ca, cx, res


FULL = Cfg(n_nodes=50000, n_edges=1600000, cores=8, f_in=128, k_clust=128,
           grp_blocks=2, cstar_blocks=180)


def kernel(x, edge_row, edge_col, edge_val, W_pool, W_embed):
    _install_profile_hook()
    x = np.asarray(x, np.float32)
    edge_row = np.asarray(edge_row, np.int32)
    edge_col = np.asarray(edge_col, np.int32)
    edge_val = np.asarray(edge_val, np.float32)

    tiles1, tiles2, planes = _prep(FULL, x, edge_row, edge_col, edge_val)
    nc = _build(FULL, tiles1, tiles2)
    ca, cx, _ = _run(FULL, nc, planes, W_pool, W_embed)
    return ca, cx


# revision 6
# speedup vs baseline: 1.5769x; 1.0414x over previous
"""DiffPool GNN kernel for one TRN2 chip (8 NeuronCores), Bass/Tile.

Math (reference):
    spmm(H) = segment_sum(edge_val[:,None] * H[edge_col], edge_row, N)
    S = softmax(relu(spmm(x @ W_pool)))         # [N, K]
    Z = relu(spmm(x @ W_embed))                 # [N, F]
    coarse_A = S.T @ spmm(S)                    # [K, K]
    coarse_X = S.T @ Z                          # [K, F]

Key reformulation: spmm(x @ W) == spmm(x) @ W, so a single SpMM Y = A@x
feeds both the pool and embed branches.  Only two SpMMs total (A@x, A@S).

Distribution: nodes are range-partitioned across the 8 cores by destination
row; each core owns the edges whose destination falls in its range.

Phase 1 (Y = A@x): edges grouped into 128-edge tiles per 128-row dest
block; host pre-gathers source rows into a dense bf16 stream Gx and builds
val-weighted one-hot planes M; TensorEngine computes Y per block as
Gx^T @ M with PSUM accumulation.  Pure streaming DMA, no gathers.

Phase 2 (A@S then S^T(A@S)): S is runtime data, so its per-edge gather
would cost ~10ns/row on the SWDGE descriptor path (measured; the serial
descriptor stage dominates at ~2ms for all edges).  Hybrid split by source
column c* balances two engines:
  - cols < c*: descriptor-path dma_gather from the all-gathered S (runs on
    the gpsimd/SWDGE ring), consumed by one-hot M2 matmuls (bf16).
  - cols >= c*: DENSE fp8 block-SpMM: host builds dense 128x128 A^T blocks
    (fp8e4, ~10.7 edges each); S blocks live SBUF-resident in fp8;
    DoubleRow matmuls contract 256 rows per instruction.  This path costs
    DMA bytes + PE time instead of serial descriptors.
The K x K / K x F coarse outputs are PSUM-accumulated per block and
all-reduced at the end.
"""

import os
import sys
import types
import importlib.util
import numpy as np
import ml_dtypes

from concourse import bass, bacc, tile, mybir, library_config
from concourse.bass_utils import run_bass_kernel_spmd

BF16 = ml_dtypes.bfloat16
F8 = ml_dtypes.float8_e4m3fn
F32 = mybir.dt.float32
DBF = mybir.dt.bfloat16
DF8 = mybir.dt.float8e4


def _install_profile_hook():
    """Register the axon NTFF profiling hook if the image's antenv lacks it."""
    try:
        import antenv.axon_hooks  # noqa: F401
        return
    except ImportError:
        pass
    try:
        spec = importlib.util.spec_from_file_location(
            "trn_boot", "/root/.axon_site/trn_agent_boot/trn_boot.py")
        trn_boot = importlib.util.module_from_spec(spec)
        spec.loader.exec_module(trn_boot)
        hook = trn_boot._ntff_profile_via_ctypes("/opt/axon/libaxon_pjrt.so")
        mod = types.ModuleType("antenv.axon_hooks")
        mod.get_axon_ntff_profile_hook = lambda: hook
        sys.modules["antenv.axon_hooks"] = mod
    except Exception:
        pass


class Cfg:
    def __init__(self, n_nodes, n_edges, cores, f_in, k_clust, grp_blocks,
                 cstar_blocks=None, split=None):
        assert n_nodes % cores == 0
        self.N = n_nodes
        self.E = n_edges
        self.CORES = cores
        self.F = f_in          # feature dim == 128
        self.K = k_clust       # cluster dim == 128
        self.PN = n_nodes // cores
        self.BLK = 128
        self.NBLK = (self.PN + 127) // 128
        self.GRP = grp_blocks
        self.NGRP = (self.NBLK + grp_blocks - 1) // grp_blocks
        # source-col blocks: total (ceil), split point for hybrid phase 2
        self.WALL = (n_nodes + 127) // 128
        if cstar_blocks is None:
            cstar_blocks = self.WALL  # all gather, no dense
        self.WSTAR = cstar_blocks
        self.CSTAR = cstar_blocks * 128
        assert self.CSTAR <= 32768  # int16 gather indices
        # dense w-blocks padded to even count for DoubleRow pairing
        self.WDN = self.WALL - self.WSTAR
        if self.WDN % 2:
            self.WDN += 1


def _sort_by_dest(cfg, edge_row, edge_col, edge_val, sel):
    """Per-core edge lists sorted by dest block; per-seg counts."""
    c = cfg
    owner = edge_row // c.PN
    cores_sorted = []
    counts = np.zeros((c.CORES, c.NBLK), np.int64)
    for m in range(c.CORES):
        s = sel & (owner == m)
        er = edge_row[s] - m * c.PN
        ec = edge_col[s]
        ev = edge_val[s]
        seg = er // c.BLK
        order = np.argsort(seg, kind="stable")
        er, ec, ev = er[order], ec[order], ev[order]
        counts[m] = np.bincount(seg[order], minlength=c.NBLK)
        cores_sorted.append((er, ec, ev))
    return cores_sorted, counts


def _tile_stream(cfg, cores_sorted, counts, want_gx, x_bf):
    """Build per-core [128,T,128] Gx/M planes + int16 idx from edge lists."""
    c = cfg
    seg_tiles = (counts.max(axis=0) + 127) // 128
    tiles_in_order = [int(t) for t in seg_tiles]
    T_total = int(sum(tiles_in_order))
    planes = []
    for m in range(c.CORES):
        er, ec, ev = cores_sorted[m]
        cnt = counts[m]
        seg_start = np.zeros(c.NBLK + 1, np.int64)
        seg_start[1:] = np.cumsum(cnt)
        cols = np.zeros(T_total * 128, np.int64)
        dest = np.zeros(T_total * 128, np.int64)
        val = np.zeros(T_total * 128, np.float32)
        pos = 0
        for b, nt in enumerate(tiles_in_order):
            a0, a1 = seg_start[b], seg_start[b + 1]
            n = a1 - a0
            cols[pos:pos + n] = ec[a0:a1]
            dest[pos:pos + n] = er[a0:a1] - b * c.BLK
            val[pos:pos + n] = ev[a0:a1]
            pos += nt * 128
        assert pos == T_total * 128
        p = {}
        if want_gx:
            gx = x_bf[cols].reshape(T_total, 128, c.F).transpose(1, 0, 2)
            p["gx"] = np.ascontiguousarray(gx)
        mv = np.zeros((T_total * 128, 128), BF16)
        mv[np.arange(T_total * 128), dest] = val.astype(BF16)
        p["mv"] = np.ascontiguousarray(
            mv.reshape(T_total, 128, 128).transpose(1, 0, 2))
        idx16 = cols.astype(np.int16)
        p["idx"] = np.tile(idx16.reshape(-1, 16).T, (8, 1)).copy()
        planes.append(p)
    return tiles_in_order, planes


def _prep(cfg, x, edge_row, edge_col, edge_val):
    c = cfg
    x_bf = np.ascontiguousarray(np.asarray(x, np.float32)).astype(BF16)
    allsel = np.ones(edge_row.shape, bool)

    # phase 1: all edges, streamed Gx + M
    cs1, cnt1 = _sort_by_dest(c, edge_row, edge_col, edge_val, allsel)
    tiles1, planes1 = _tile_stream(c, cs1, cnt1, True, x_bf)

    # phase 2 gather path: cols < CSTAR
    gsel = edge_col < c.CSTAR
    cs2, cnt2 = _sort_by_dest(c, edge_row, edge_col, edge_val, gsel)
    tiles2, planes2 = _tile_stream(c, cs2, cnt2, False, None)

    # phase 2 dense path: cols >= CSTAR -> fp8 A^T blocks
    # layout per core: [NBLK, 128 j, WDN*128 d]; A8[b, j, (w-WSTAR)*128+d]
    planesA = []
    if c.WDN:
        owner = edge_row // c.PN
        for m in range(c.CORES):
            s = (~gsel) & (owner == m)
            er = (edge_row[s] - m * c.PN).astype(np.int64)
            ec = edge_col[s].astype(np.int64)
            ev = edge_val[s].astype(np.float32)
            b = er // 128
            d = er % 128
            w = ec // 128 - c.WSTAR
            j = ec % 128
            a = np.zeros(c.NBLK * 128 * c.WDN * 128, np.float32)
            flat = ((b * 128 + j) * c.WDN + w) * 128 + d
            np.add.at(a, flat, ev)
            planesA.append(
                a.reshape(c.NBLK, 128, c.WDN * 128).astype(F8))
    else:
        planesA = [np.zeros((c.NBLK, 128, 0), F8) for _ in range(c.CORES)]

    planes = []
    for m in range(c.CORES):
        p = {"gx": planes1[m]["gx"], "mv": planes1[m]["mv"],
             "mv2": planes2[m]["mv"], "idx2": planes2[m]["idx"],
             "ab": planesA[m]}
        planes.append(p)
    return tiles1, tiles2, planes


def _mk_groups(cfg, tiles_in_order):
    call_info = []
    pos = 0
    for g in range(cfg.NGRP):
        bl = range(g * cfg.GRP, min(cfg.NBLK, (g + 1) * cfg.GRP))
        off = 0
        entries = []
        for b in bl:
            nt = tiles_in_order[b]
            entries.append((b, off, nt))
            off += nt
        call_info.append((pos, entries))
        pos += off
    grpmax = max(sum(nt for _, _, nt in e) for _, e in call_info)
    return call_info, max(grpmax, 1)


def _build(cfg, tiles1, tiles2):
    c = cfg
    T1 = sum(tiles1)
    T2 = max(sum(tiles2), 1)
    ci1, GM1 = _mk_groups(c, tiles1)
    ci2, GM2 = _mk_groups(c, tiles2)
    GM = max(GM1, GM2)

    nc = bacc.Bacc("TRN2", target_bir_lowering=False, debug=False,
                   num_devices=c.CORES)
    gx_d = nc.dram_tensor("gx", [128, T1, c.F], DBF, kind="ExternalInput").ap()
    mv_d = nc.dram_tensor("mv", [128, T1, 128], DBF, kind="ExternalInput").ap()
    mv2_d = nc.dram_tensor("mv2", [128, T2, 128], DBF,
                           kind="ExternalInput").ap()
    idx2_d = nc.dram_tensor("idx2", [128, T2 * 8], mybir.dt.int16,
                            kind="ExternalInput").ap()
    ab_d = nc.dram_tensor("ab", [c.NBLK, 128, max(c.WDN, 1) * 128], DF8,
                          kind="ExternalInput").ap()
    rmask_d = nc.dram_tensor("rmask", [128, 1], F32, kind="ExternalInput").ap()
    wp_d = nc.dram_tensor("wp", [c.F, c.K], DBF, kind="ExternalInput").ap()
    we_d = nc.dram_tensor("we", [c.F, c.K], DBF, kind="ExternalInput").ap()
    ca_d = nc.dram_tensor("coarse_A", [c.K, c.K], F32,
                          kind="ExternalOutput").ap()
    cx_d = nc.dram_tensor("coarse_X", [c.K, c.F], F32,
                          kind="ExternalOutput").ap()

    s_bounce = nc.dram_tensor("s_bounce", [c.PN, c.K], DBF).ap()
    s_full = nc.dram_tensor("s_full", [c.N, c.K], DBF).ap()
    cc_in = nc.dram_tensor("cc_in", [128, 256], F32).ap()
    cc_out = nc.dram_tensor("cc_out", [128, 256], F32).ap()

    last_rows = c.PN - (c.NBLK - 1) * c.BLK

    with tile.TileContext(nc) as tc:
        with (
            tc.tile_pool(name="const", bufs=1) as constp,
            tc.tile_pool(name="gbuf", bufs=2) as gpool,
            tc.tile_pool(name="mbuf", bufs=2) as mpool,
            tc.tile_pool(name="node", bufs=1) as nodep,
            tc.tile_pool(name="small", bufs=4) as smallp,
            tc.tile_pool(name="abuf", bufs=3) as apool,
            tc.tile_pool(name="s8p", bufs=1) as s8pool,
            tc.tile_pool(name="py", bufs=4, space="PSUM") as psum_y,
            tc.tile_pool(name="pde", bufs=2, space="PSUM") as psum_de,
            tc.tile_pool(name="pca", bufs=1, space="PSUM") as psum_ca,
            tc.tile_pool(name="pcx", bufs=1, space="PSUM") as psum_cx,
        ):
            nc.gpsimd.load_library(library_config.mlp)

            idx_sb = constp.tile([128, T2 * 8], mybir.dt.int16)
            rmask_sb = constp.tile([128, 1], F32)
            wp_sb = constp.tile([c.F, c.K], DBF)
            we_sb = constp.tile([c.F, c.K], DBF)
            nc.sync.dma_start(out=idx_sb[:, :], in_=idx2_d[:, :])
            nc.sync.dma_start(out=rmask_sb[:, :], in_=rmask_d[:, :])
            nc.sync.dma_start(out=wp_sb[:, :], in_=wp_d[:, :])
            nc.sync.dma_start(out=we_sb[:, :], in_=we_d[:, :])

            yt_sb = nodep.tile([128, c.NBLK, 128], DBF)
            s_sb = nodep.tile([128, c.NBLK, c.K], DBF)
            z_sb = nodep.tile([128, c.NBLK, c.K], DBF)

            p_ca = psum_ca.tile([128, c.K], F32)
            p_cx = psum_cx.tile([128, c.K], F32)

            # ---- SpMM-1 + dense + softmax --------------------------------
            for gi in range(c.NGRP):
                g0, entries = ci1[gi]
                tg = sum(nt for _, _, nt in entries)
                if tg == 0:
                    continue
                mb = mpool.tile([128, GM, 128], DBF, tag="mb")
                nc.scalar.dma_start(out=mb[:, 0:tg, :],
                                    in_=mv_d[:, g0:g0 + tg, :])
                gb = gpool.tile([128, GM, 128], DBF, tag="gb")
                nc.sync.dma_start(out=gb[:, 0:tg, :],
                                  in_=gx_d[:, g0:g0 + tg, :])
                for (b, off, nt) in entries:
                    if nt == 0:
                        continue
                    acc = psum_y.tile([128, 128], F32, tag="py")
                    for t in range(nt):
                        nc.tensor.matmul(acc[:, :], gb[:, off + t, :],
                                         mb[:, off + t, :],
                                         start=(t == 0), stop=(t == nt - 1))
                    nc.vector.tensor_copy(yt_sb[:, b, :], acc[:, :])
                    pde = psum_de.tile([128, 2, 128], F32, tag="pde")
                    nc.tensor.matmul(pde[:, 0, :], yt_sb[:, b, :], wp_sb[:, :])
                    nc.tensor.matmul(pde[:, 1, :], yt_sb[:, b, :], we_sb[:, :])
                    lg = smallp.tile([128, 128], F32, tag="lg")
                    mx = smallp.tile([128, 1], F32, tag="mx")
                    ex = smallp.tile([128, 128], F32, tag="ex")
                    sm = smallp.tile([128, 1], F32, tag="sm")
                    rc = smallp.tile([128, 1], F32, tag="rc")
                    nc.vector.tensor_scalar_max(lg[:, :], pde[:, 0, :], 0.0)
                    nc.vector.tensor_reduce(mx[:, :], lg[:, :],
                                            axis=mybir.AxisListType.X,
                                            op=mybir.AluOpType.max,
                                            negate=True)
                    nc.scalar.activation(ex[:, :], lg[:, :],
                                         mybir.ActivationFunctionType.Exp,
                                         bias=mx[:, 0:1], scale=1.0,
                                         accum_out=sm[:, 0:1])
                    nc.vector.reciprocal(rc[:, :], sm[:, :])
                    if b == c.NBLK - 1 and last_rows < 128:
                        nc.vector.tensor_scalar(
                            out=s_sb[:, b, :], in0=ex[:, :],
                            scalar1=rc[:, 0:1], scalar2=rmask_sb[:, 0:1],
                            op0=mybir.AluOpType.mult,
                            op1=mybir.AluOpType.mult)
                        nc.vector.tensor_scalar(
                            out=z_sb[:, b, :], in0=pde[:, 1, :], scalar1=0.0,
                            scalar2=rmask_sb[:, 0:1],
                            op0=mybir.AluOpType.max,
                            op1=mybir.AluOpType.mult)
                    else:
                        nc.vector.tensor_scalar_mul(s_sb[:, b, :], ex[:, :],
                                                    rc[:, 0:1])
                        nc.vector.tensor_scalar_max(z_sb[:, b, :],
                                                    pde[:, 1, :], 0.0)
                    nc.tensor.matmul(p_cx[:, :], s_sb[:, b, :], z_sb[:, b, :],
                                     start=(b == 0), stop=(b == c.NBLK - 1))

            # ---- AllGather S ---------------------------------------------
            nfull = c.NBLK - 1
            nc.sync.dma_start(
                out=s_bounce[0:nfull * 128, :].rearrange(
                    "(b p) k -> p b k", p=128),
                in_=s_sb[:, 0:nfull, :])
            nc.sync.dma_start(
                out=s_bounce[nfull * 128:c.PN, :],
                in_=s_sb[0:last_rows, nfull, :])
            nc.gpsimd.collective_compute(
                "AllGather", mybir.AluOpType.bypass,
                replica_groups=[list(range(c.CORES))],
                ins=[s_bounce.opt()], outs=[s_full.opt()])

            # ---- S blocks >= CSTAR resident in SBUF as fp8 ---------------
            if c.WDN:
                s8 = s8pool.tile([128, c.WDN, 128], DF8)
                nc.vector.memset(s8[:, :, :], 0.0)
                CH = 8
                for w0 in range(0, c.WDN, CH):
                    wn = min(CH, c.WDN - w0)
                    r0 = (c.WSTAR + w0) * 128
                    rn = min(wn * 128, c.N - r0)
                    if rn <= 0:
                        break
                    tmp = smallp.tile([128, CH, 128], DBF, tag="scast")
                    full = rn // 128
                    if full:
                        nc.sync.dma_start(
                            out=tmp[:, 0:full, :],
                            in_=s_full[r0:r0 + full * 128, :].rearrange(
                                "(a p) k -> p a k", p=128))
                    rem = rn - full * 128
                    if rem > 0:
                        nc.vector.memset(tmp[:, full, :], 0.0)
                        nc.sync.dma_start(
                            out=tmp[0:rem, full, :],
                            in_=s_full[r0 + full * 128:r0 + rn, :])
                    nw = full + (1 if rem else 0)
                    nc.vector.tensor_copy(s8[:, w0:w0 + nw, :],
                                          tmp[:, 0:nw, :])

            # ---- SpMM-2 (A @ S): dense fp8 + gathered bf16 ---------------
            for gi in range(c.NGRP):
                g0, entries = ci2[gi]
                tg = sum(nt for _, _, nt in entries)
                mb = mpool.tile([128, GM, 128], DBF, tag="mb")
                if tg:
                    nc.scalar.dma_start(out=mb[:, 0:tg, :],
                                        in_=mv2_d[:, g0:g0 + tg, :])
                gb = gpool.tile([128, GM, 128], DBF, tag="gb")
                if tg:
                    nc.gpsimd.dma_gather(
                        out_ap=gb[:, 0:tg, :],
                        in_ap=s_full[0:c.CSTAR, :],
                        idxs_ap=idx_sb[:, g0 * 8:(g0 + tg) * 8],
                        num_idxs=tg * 128, num_idxs_reg=tg * 128,
                        elem_size=c.F, single_packet=False)
                for (b, off, nt) in entries:
                    acc = psum_y.tile([128, 128], F32, tag="py")
                    total = (c.WDN // 2 if c.WDN else 0) + nt
                    if total == 0:
                        continue
                    done = 0
                    ACH = 32
                    if c.WDN:
                        for w0 in range(0, c.WDN, ACH):
                            wn = min(ACH, c.WDN - w0)
                            abt = apool.tile([128, ACH, 128], DF8, tag="ab")
                            nc.sync.dma_start(
                                out=abt[:, 0:wn, :],
                                in_=ab_d[b, :, w0 * 128:(w0 + wn) * 128]
                                .rearrange("p (w d) -> p w d", d=128))
                            for wp in range(0, wn, 2):
                                done += 1
                                nc.tensor.matmul(
                                    acc[:, :], abt[:, wp:wp + 2, :],
                                    s8[:, w0 + wp:w0 + wp + 2, :],
                                    start=(done == 1), stop=(done == total),
                                    perf_mode=mybir.MatmulPerfMode.DoubleRow)
                    for t in range(nt):
                        done += 1
                        nc.tensor.matmul(acc[:, :], mb[:, off + t, :],
                                         gb[:, off + t, :],
                                         start=(done == 1),
                                         stop=(done == total))
                    asb = smallp.tile([128, 128], DBF, tag="asb")
                    nc.vector.tensor_copy(asb[:, :], acc[:, :])
                    nc.tensor.matmul(p_ca[:, :], s_sb[:, b, :], asb[:, :],
                                     start=(b == 0), stop=(b == c.NBLK - 1))

            # ---- AllReduce + outputs -------------------------------------
            cc_sb = smallp.tile([128, 256], F32, tag="cc")
            nc.vector.tensor_copy(cc_sb[:, 0:128], p_ca[:, :])
            nc.vector.tensor_copy(cc_sb[:, 128:256], p_cx[:, :])
            nc.sync.dma_start(out=cc_in[:, :], in_=cc_sb[:, :])
            nc.gpsimd.collective_compute(
                "AllReduce", mybir.AluOpType.add,
                replica_groups=[list(range(c.CORES))],
                ins=[cc_in.opt()], outs=[cc_out.opt()])
            out_sb = smallp.tile([128, 256], F32, tag="cc")
            nc.sync.dma_start(out=out_sb[:, :], in_=cc_out[:, :])
            nc.sync.dma_start(out=ca_d[:, :], in_=out_sb[:, 0:128])
            nc.sync.dma_start(out=cx_d[:, :], in_=out_sb[:, 128:256])

    nc.compile()
    return nc


def _run(cfg, nc, planes, W_pool, W_embed, trace=False):
    c = cfg
    rmask = np.zeros((128, 1), np.float32)
    lr = c.PN - (c.NBLK - 1) * 128 if c.PN % 128 else 128
    rmask[:lr] = 1.0
    wp = np.ascontiguousarray(np.asarray(W_pool, np.float32)).astype(BF16)
    we = np.ascontiguousarray(np.asarray(W_embed, np.float32)).astype(BF16)
    in_maps = []
    for m in range(c.CORES):
        in_maps.append({
            "rmask": rmask, "wp": wp, "we": we,
            "gx": planes[m]["gx"], "mv": planes[m]["mv"],
            "mv2": planes[m]["mv2"], "idx2": planes[m]["idx2"],
            "ab": planes[m]["ab"],
        })
    res = run_bass_kernel_spmd(nc, in_maps, list(range(c.CORES)), trace=trace)
    ca = np.asarray(res.results[0]["coarse_A"], np.float32)
    cx = np.asarray(res.results[0]["coarse_X"], np.float32)
    return ca, cx, res


FULL = Cfg(n_nodes=50000, n_edges=1600000, cores=8, f_in=128, k_clust=128,
           grp_blocks=2, cstar_blocks=245)


def kernel(x, edge_row, edge_col, edge_val, W_pool, W_embed):
    _install_profile_hook()
    x = np.asarray(x, np.float32)
    edge_row = np.asarray(edge_row, np.int32)
    edge_col = np.asarray(edge_col, np.int32)
    edge_val = np.asarray(edge_val, np.float32)

    tiles1, tiles2, planes = _prep(FULL, x, edge_row, edge_col, edge_val)
    nc = _build(FULL, tiles1, tiles2)
    ca, cx, _ = _run(FULL, nc, planes, W_pool, W_embed)
    return ca, cx


# revision 7
# speedup vs baseline: 1.5960x; 1.0121x over previous
"""DiffPool GNN kernel for one TRN2 chip (8 NeuronCores), Bass/Tile.

Math (reference):
    spmm(H) = segment_sum(edge_val[:,None] * H[edge_col], edge_row, N)
    S = softmax(relu(spmm(x @ W_pool)))         # [N, K]
    Z = relu(spmm(x @ W_embed))                 # [N, F]
    coarse_A = S.T @ spmm(S)                    # [K, K]
    coarse_X = S.T @ Z                          # [K, F]

Key reformulation: spmm(x @ W) == spmm(x) @ W, so a single SpMM Y = A@x
feeds both the pool and embed branches.  Only two SpMMs total (A@x, A@S).

Distribution: nodes are range-partitioned across the 8 cores by destination
row; each core owns the edges whose destination falls in its range.

Phase 1 (Y = A@x): edges grouped into 128-edge tiles per 128-row dest
block; host pre-gathers source rows into a dense bf16 stream Gx and builds
val-weighted one-hot planes M; TensorEngine computes Y per block as
Gx^T @ M with PSUM accumulation.  Pure streaming DMA, no gathers.

Phase 2 (A@S then S^T(A@S)): S is runtime data, so its per-edge gather
would cost ~10ns/row on the SWDGE descriptor path (measured; the serial
descriptor stage dominates at ~2ms for all edges).  Hybrid split by source
column c* balances two engines:
  - cols < c*: descriptor-path dma_gather from the all-gathered S (runs on
    the gpsimd/SWDGE ring), consumed by one-hot M2 matmuls (bf16).
  - cols >= c*: DENSE fp8 block-SpMM: host builds dense 128x128 A^T blocks
    (fp8e4, ~10.7 edges each); S blocks live SBUF-resident in fp8;
    DoubleRow matmuls contract 256 rows per instruction.  This path costs
    DMA bytes + PE time instead of serial descriptors.
The K x K / K x F coarse outputs are PSUM-accumulated per block and
all-reduced at the end.
"""

import os
import sys
import types
import importlib.util
import numpy as np
import ml_dtypes

from concourse import bass, bacc, tile, mybir, library_config
from concourse.bass_utils import run_bass_kernel_spmd

BF16 = ml_dtypes.bfloat16
F8 = ml_dtypes.float8_e4m3fn
F32 = mybir.dt.float32
DBF = mybir.dt.bfloat16
DF8 = mybir.dt.float8e4


def _install_profile_hook():
    """Register the axon NTFF profiling hook if the image's antenv lacks it."""
    try:
        import antenv.axon_hooks  # noqa: F401
        return
    except ImportError:
        pass
    try:
        spec = importlib.util.spec_from_file_location(
            "trn_boot", "/root/.axon_site/trn_agent_boot/trn_boot.py")
        trn_boot = importlib.util.module_from_spec(spec)
        spec.loader.exec_module(trn_boot)
        hook = trn_boot._ntff_profile_via_ctypes("/opt/axon/libaxon_pjrt.so")
        mod = types.ModuleType("antenv.axon_hooks")
        mod.get_axon_ntff_profile_hook = lambda: hook
        sys.modules["antenv.axon_hooks"] = mod
    except Exception:
        pass


class Cfg:
    def __init__(self, n_nodes, n_edges, cores, f_in, k_clust, grp_blocks,
                 cstar_blocks=None, split=None):
        assert n_nodes % cores == 0
        self.N = n_nodes
        self.E = n_edges
        self.CORES = cores
        self.F = f_in          # feature dim == 128
        self.K = k_clust       # cluster dim == 128
        self.PN = n_nodes // cores
        self.BLK = 128
        self.NBLK = (self.PN + 127) // 128
        self.GRP = grp_blocks
        self.NGRP = (self.NBLK + grp_blocks - 1) // grp_blocks
        # source-col blocks: total (ceil), split point for hybrid phase 2
        self.WALL = (n_nodes + 127) // 128
        if cstar_blocks is None:
            cstar_blocks = self.WALL  # all gather, no dense
        self.WSTAR = cstar_blocks
        self.CSTAR = cstar_blocks * 128
        assert self.CSTAR <= 32768  # int16 gather indices
        # dense w-blocks padded to even count for DoubleRow pairing
        self.WDN = self.WALL - self.WSTAR
        if self.WDN % 2:
            self.WDN += 1


def _sort_by_dest(cfg, edge_row, edge_col, edge_val, sel):
    """Per-core edge lists sorted by dest block; per-seg counts."""
    c = cfg
    owner = edge_row // c.PN
    cores_sorted = []
    counts = np.zeros((c.CORES, c.NBLK), np.int64)
    for m in range(c.CORES):
        s = sel & (owner == m)
        er = edge_row[s] - m * c.PN
        ec = edge_col[s]
        ev = edge_val[s]
        seg = er // c.BLK
        order = np.argsort(seg, kind="stable")
        er, ec, ev = er[order], ec[order], ev[order]
        counts[m] = np.bincount(seg[order], minlength=c.NBLK)
        cores_sorted.append((er, ec, ev))
    return cores_sorted, counts


def _tile_stream(cfg, cores_sorted, counts, want_gx, x_bf):
    """Build per-core [128,T,128] Gx/M planes + int16 idx from edge lists."""
    c = cfg
    seg_tiles = (counts.max(axis=0) + 127) // 128
    tiles_in_order = [int(t) for t in seg_tiles]
    T_total = int(sum(tiles_in_order))
    planes = []
    for m in range(c.CORES):
        er, ec, ev = cores_sorted[m]
        cnt = counts[m]
        seg_start = np.zeros(c.NBLK + 1, np.int64)
        seg_start[1:] = np.cumsum(cnt)
        cols = np.zeros(T_total * 128, np.int64)
        dest = np.zeros(T_total * 128, np.int64)
        val = np.zeros(T_total * 128, np.float32)
        pos = 0
        for b, nt in enumerate(tiles_in_order):
            a0, a1 = seg_start[b], seg_start[b + 1]
            n = a1 - a0
            cols[pos:pos + n] = ec[a0:a1]
            dest[pos:pos + n] = er[a0:a1] - b * c.BLK
            val[pos:pos + n] = ev[a0:a1]
            pos += nt * 128
        assert pos == T_total * 128
        p = {}
        if want_gx:
            # fold edge_val into the gathered x rows; stream fp8
            gx = (x_bf[cols].astype(np.float32)
                  * val[:, None]).astype(F8)
            gx = gx.reshape(T_total, 128, c.F).transpose(1, 0, 2)
            p["gx"] = np.ascontiguousarray(gx)
            mv = np.zeros((T_total * 128, 128), F8)
            mv[np.arange(T_total * 128), dest] = np.where(
                val != 0.0, np.float32(1.0), np.float32(0.0)).astype(F8)
            p["mv"] = np.ascontiguousarray(
                mv.reshape(T_total, 128, 128).transpose(1, 0, 2))
        else:
            mv = np.zeros((T_total * 128, 128), BF16)
            mv[np.arange(T_total * 128), dest] = val.astype(BF16)
            p["mv"] = np.ascontiguousarray(
                mv.reshape(T_total, 128, 128).transpose(1, 0, 2))
        idx16 = cols.astype(np.int16)
        p["idx"] = np.tile(idx16.reshape(-1, 16).T, (8, 1)).copy()
        planes.append(p)
    return tiles_in_order, planes


def _prep(cfg, x, edge_row, edge_col, edge_val):
    c = cfg
    x_bf = np.ascontiguousarray(np.asarray(x, np.float32)).astype(BF16)
    allsel = np.ones(edge_row.shape, bool)

    # phase 1: all edges, streamed Gx + M
    cs1, cnt1 = _sort_by_dest(c, edge_row, edge_col, edge_val, allsel)
    tiles1, planes1 = _tile_stream(c, cs1, cnt1, True, x_bf)

    # phase 2 gather path: cols < CSTAR
    gsel = edge_col < c.CSTAR
    cs2, cnt2 = _sort_by_dest(c, edge_row, edge_col, edge_val, gsel)
    tiles2, planes2 = _tile_stream(c, cs2, cnt2, False, None)

    # phase 2 dense path: cols >= CSTAR -> fp8 A^T blocks
    # layout per core: [NBLK, 128 j, WDN*128 d]; A8[b, j, (w-WSTAR)*128+d]
    planesA = []
    if c.WDN:
        owner = edge_row // c.PN
        for m in range(c.CORES):
            s = (~gsel) & (owner == m)
            er = (edge_row[s] - m * c.PN).astype(np.int64)
            ec = edge_col[s].astype(np.int64)
            ev = edge_val[s].astype(np.float32)
            b = er // 128
            d = er % 128
            w = ec // 128 - c.WSTAR
            j = ec % 128
            a = np.zeros(c.NBLK * 128 * c.WDN * 128, np.float32)
            flat = ((b * 128 + j) * c.WDN + w) * 128 + d
            np.add.at(a, flat, ev)
            planesA.append(
                a.reshape(c.NBLK, 128, c.WDN * 128).astype(F8))
    else:
        planesA = [np.zeros((c.NBLK, 128, 0), F8) for _ in range(c.CORES)]

    planes = []
    for m in range(c.CORES):
        p = {"gx": planes1[m]["gx"], "mv": planes1[m]["mv"],
             "mv2": planes2[m]["mv"], "idx2": planes2[m]["idx"],
             "ab": planesA[m]}
        planes.append(p)
    return tiles1, tiles2, planes


def _mk_groups(cfg, tiles_in_order):
    call_info = []
    pos = 0
    for g in range(cfg.NGRP):
        bl = range(g * cfg.GRP, min(cfg.NBLK, (g + 1) * cfg.GRP))
        off = 0
        entries = []
        for b in bl:
            nt = tiles_in_order[b]
            entries.append((b, off, nt))
            off += nt
        call_info.append((pos, entries))
        pos += off
    grpmax = max(sum(nt for _, _, nt in e) for _, e in call_info)
    return call_info, max(grpmax, 1)


def _build(cfg, tiles1, tiles2):
    c = cfg
    T1 = sum(tiles1)
    T2 = max(sum(tiles2), 1)
    ci1, GM1 = _mk_groups(c, tiles1)
    ci2, GM2 = _mk_groups(c, tiles2)
    GM = max(GM1, GM2)

    nc = bacc.Bacc("TRN2", target_bir_lowering=False, debug=False,
                   num_devices=c.CORES)
    gx_d = nc.dram_tensor("gx", [128, T1, c.F], DF8, kind="ExternalInput").ap()
    mv_d = nc.dram_tensor("mv", [128, T1, 128], DF8, kind="ExternalInput").ap()
    mv2_d = nc.dram_tensor("mv2", [128, T2, 128], DBF,
                           kind="ExternalInput").ap()
    idx2_d = nc.dram_tensor("idx2", [128, T2 * 8], mybir.dt.int16,
                            kind="ExternalInput").ap()
    ab_d = nc.dram_tensor("ab", [c.NBLK, 128, max(c.WDN, 1) * 128], DF8,
                          kind="ExternalInput").ap()
    rmask_d = nc.dram_tensor("rmask", [128, 1], F32, kind="ExternalInput").ap()
    wp_d = nc.dram_tensor("wp", [c.F, c.K], DBF, kind="ExternalInput").ap()
    we_d = nc.dram_tensor("we", [c.F, c.K], DBF, kind="ExternalInput").ap()
    ca_d = nc.dram_tensor("coarse_A", [c.K, c.K], F32,
                          kind="ExternalOutput").ap()
    cx_d = nc.dram_tensor("coarse_X", [c.K, c.F], F32,
                          kind="ExternalOutput").ap()

    s_bounce = nc.dram_tensor("s_bounce", [c.PN, c.K], DBF).ap()
    s_full = nc.dram_tensor("s_full", [c.N, c.K], DBF).ap()
    cc_in = nc.dram_tensor("cc_in", [128, 256], F32).ap()
    cc_out = nc.dram_tensor("cc_out", [128, 256], F32).ap()

    last_rows = c.PN - (c.NBLK - 1) * c.BLK

    with tile.TileContext(nc) as tc:
        with (
            tc.tile_pool(name="const", bufs=1) as constp,
            tc.tile_pool(name="gbuf", bufs=2) as gpool,
            tc.tile_pool(name="mbuf", bufs=2) as mpool,
            tc.tile_pool(name="node", bufs=1) as nodep,
            tc.tile_pool(name="small", bufs=4) as smallp,
            tc.tile_pool(name="abuf", bufs=3) as apool,
            tc.tile_pool(name="s8p", bufs=1) as s8pool,
            tc.tile_pool(name="py", bufs=4, space="PSUM") as psum_y,
            tc.tile_pool(name="pde", bufs=2, space="PSUM") as psum_de,
            tc.tile_pool(name="pca", bufs=1, space="PSUM") as psum_ca,
            tc.tile_pool(name="pcx", bufs=1, space="PSUM") as psum_cx,
        ):
            nc.gpsimd.load_library(library_config.mlp)

            idx_sb = constp.tile([128, T2 * 8], mybir.dt.int16)
            rmask_sb = constp.tile([128, 1], F32)
            wp_sb = constp.tile([c.F, c.K], DBF)
            we_sb = constp.tile([c.F, c.K], DBF)
            nc.sync.dma_start(out=idx_sb[:, :], in_=idx2_d[:, :])
            nc.sync.dma_start(out=rmask_sb[:, :], in_=rmask_d[:, :])
            nc.sync.dma_start(out=wp_sb[:, :], in_=wp_d[:, :])
            nc.sync.dma_start(out=we_sb[:, :], in_=we_d[:, :])

            yt_sb = nodep.tile([128, c.NBLK, 128], DBF)
            s_sb = nodep.tile([128, c.NBLK, c.K], DBF)
            z_sb = nodep.tile([128, c.NBLK, c.K], DBF)

            p_ca = psum_ca.tile([128, c.K], F32)
            p_cx = psum_cx.tile([128, c.K], F32)

            # ---- SpMM-1 + dense + softmax --------------------------------
            for gi in range(c.NGRP):
                g0, entries = ci1[gi]
                tg = sum(nt for _, _, nt in entries)
                if tg == 0:
                    continue
                mb = mpool.tile([128, GM, 128], DF8, tag="mb8")
                nc.scalar.dma_start(out=mb[:, 0:tg, :],
                                    in_=mv_d[:, g0:g0 + tg, :])
                gb = gpool.tile([128, GM, 128], DF8, tag="gb8")
                nc.sync.dma_start(out=gb[:, 0:tg, :],
                                  in_=gx_d[:, g0:g0 + tg, :])
                for (b, off, nt) in entries:
                    if nt == 0:
                        continue
                    acc = psum_y.tile([128, 128], F32, tag="py")
                    for t in range(nt):
                        nc.tensor.matmul(acc[:, :], gb[:, off + t, :],
                                         mb[:, off + t, :],
                                         start=(t == 0), stop=(t == nt - 1))
                    nc.vector.tensor_copy(yt_sb[:, b, :], acc[:, :])
                    pde = psum_de.tile([128, 2, 128], F32, tag="pde")
                    nc.tensor.matmul(pde[:, 0, :], yt_sb[:, b, :], wp_sb[:, :])
                    nc.tensor.matmul(pde[:, 1, :], yt_sb[:, b, :], we_sb[:, :])
                    lg = smallp.tile([128, 128], F32, tag="lg")
                    mx = smallp.tile([128, 1], F32, tag="mx")
                    ex = smallp.tile([128, 128], F32, tag="ex")
                    sm = smallp.tile([128, 1], F32, tag="sm")
                    rc = smallp.tile([128, 1], F32, tag="rc")
                    nc.vector.tensor_scalar_max(lg[:, :], pde[:, 0, :], 0.0)
                    nc.vector.tensor_reduce(mx[:, :], lg[:, :],
                                            axis=mybir.AxisListType.X,
                                            op=mybir.AluOpType.max,
                                            negate=True)
                    nc.scalar.activation(ex[:, :], lg[:, :],
                                         mybir.ActivationFunctionType.Exp,
                                         bias=mx[:, 0:1], scale=1.0,
                                         accum_out=sm[:, 0:1])
                    nc.vector.reciprocal(rc[:, :], sm[:, :])
                    if b == c.NBLK - 1 and last_rows < 128:
                        nc.vector.tensor_scalar(
                            out=s_sb[:, b, :], in0=ex[:, :],
                            scalar1=rc[:, 0:1], scalar2=rmask_sb[:, 0:1],
                            op0=mybir.AluOpType.mult,
                            op1=mybir.AluOpType.mult)
                        nc.vector.tensor_scalar(
                            out=z_sb[:, b, :], in0=pde[:, 1, :], scalar1=0.0,
                            scalar2=rmask_sb[:, 0:1],
                            op0=mybir.AluOpType.max,
                            op1=mybir.AluOpType.mult)
                    else:
                        nc.vector.tensor_scalar_mul(s_sb[:, b, :], ex[:, :],
                                                    rc[:, 0:1])
                        nc.vector.tensor_scalar_max(z_sb[:, b, :],
                                                    pde[:, 1, :], 0.0)
                    nc.tensor.matmul(p_cx[:, :], s_sb[:, b, :], z_sb[:, b, :],
                                     start=(b == 0), stop=(b == c.NBLK - 1))

            # ---- AllGather S ---------------------------------------------
            nfull = c.NBLK - 1
            nc.sync.dma_start(
                out=s_bounce[0:nfull * 128, :].rearrange(
                    "(b p) k -> p b k", p=128),
                in_=s_sb[:, 0:nfull, :])
            nc.sync.dma_start(
                out=s_bounce[nfull * 128:c.PN, :],
                in_=s_sb[0:last_rows, nfull, :])
            nc.gpsimd.collective_compute(
                "AllGather", mybir.AluOpType.bypass,
                replica_groups=[list(range(c.CORES))],
                ins=[s_bounce.opt()], outs=[s_full.opt()])

            # ---- S blocks >= CSTAR resident in SBUF as fp8 ---------------
            if c.WDN:
                s8 = s8pool.tile([128, c.WDN, 128], DF8)
                nc.vector.memset(s8[:, :, :], 0.0)
                CH = 8
                for w0 in range(0, c.WDN, CH):
                    wn = min(CH, c.WDN - w0)
                    r0 = (c.WSTAR + w0) * 128
                    rn = min(wn * 128, c.N - r0)
                    if rn <= 0:
                        break
                    tmp = smallp.tile([128, CH, 128], DBF, tag="scast")
                    full = rn // 128
                    if full:
                        nc.sync.dma_start(
                            out=tmp[:, 0:full, :],
                            in_=s_full[r0:r0 + full * 128, :].rearrange(
                                "(a p) k -> p a k", p=128))
                    rem = rn - full * 128
                    if rem > 0:
                        nc.vector.memset(tmp[:, full, :], 0.0)
                        nc.sync.dma_start(
                            out=tmp[0:rem, full, :],
                            in_=s_full[r0 + full * 128:r0 + rn, :])
                    nw = full + (1 if rem else 0)
                    nc.vector.tensor_copy(s8[:, w0:w0 + nw, :],
                                          tmp[:, 0:nw, :])

            # ---- SpMM-2 (A @ S): dense fp8 + gathered bf16 ---------------
            for gi in range(c.NGRP):
                g0, entries = ci2[gi]
                tg = sum(nt for _, _, nt in entries)
                mb = mpool.tile([128, GM, 128], DBF, tag="mb")
                if tg:
                    nc.scalar.dma_start(out=mb[:, 0:tg, :],
                                        in_=mv2_d[:, g0:g0 + tg, :])
                gb = gpool.tile([128, GM, 128], DBF, tag="gb")
                if tg:
                    nc.gpsimd.dma_gather(
                        out_ap=gb[:, 0:tg, :],
                        in_ap=s_full[0:c.CSTAR, :],
                        idxs_ap=idx_sb[:, g0 * 8:(g0 + tg) * 8],
                        num_idxs=tg * 128, num_idxs_reg=tg * 128,
                        elem_size=c.F, single_packet=False)
                for (b, off, nt) in entries:
                    acc = psum_y.tile([128, 128], F32, tag="py")
                    total = (c.WDN // 2 if c.WDN else 0) + nt
                    if total == 0:
                        continue
                    done = 0
                    ACH = 32
                    if c.WDN:
                        for w0 in range(0, c.WDN, ACH):
                            wn = min(ACH, c.WDN - w0)
                            abt = apool.tile([128, ACH, 128], DF8, tag="ab")
                            nc.sync.dma_start(
                                out=abt[:, 0:wn, :],
                                in_=ab_d[b, :, w0 * 128:(w0 + wn) * 128]
                                .rearrange("p (w d) -> p w d", d=128))
                            for wp in range(0, wn, 2):
                                done += 1
                                nc.tensor.matmul(
                                    acc[:, :], abt[:, wp:wp + 2, :],
                                    s8[:, w0 + wp:w0 + wp + 2, :],
                                    start=(done == 1), stop=(done == total),
                                    perf_mode=mybir.MatmulPerfMode.DoubleRow)
                    for t in range(nt):
                        done += 1
                        nc.tensor.matmul(acc[:, :], mb[:, off + t, :],
                                         gb[:, off + t, :],
                                         start=(done == 1),
                                         stop=(done == total))
                    asb = smallp.tile([128, 128], DBF, tag="asb")
                    nc.vector.tensor_copy(asb[:, :], acc[:, :])
                    nc.tensor.matmul(p_ca[:, :], s_sb[:, b, :], asb[:, :],
                                     start=(b == 0), stop=(b == c.NBLK - 1))

            # ---- AllReduce + outputs -------------------------------------
            cc_sb = smallp.tile([128, 256], F32, tag="cc")
            nc.vector.tensor_copy(cc_sb[:, 0:128], p_ca[:, :])
            nc.vector.tensor_copy(cc_sb[:, 128:256], p_cx[:, :])
            nc.sync.dma_start(out=cc_in[:, :], in_=cc_sb[:, :])
            nc.gpsimd.collective_compute(
                "AllReduce", mybir.AluOpType.add,
                replica_groups=[list(range(c.CORES))],
                ins=[cc_in.opt()], outs=[cc_out.opt()])
            out_sb = smallp.tile([128, 256], F32, tag="cc")
            nc.sync.dma_start(out=out_sb[:, :], in_=cc_out[:, :])
            nc.sync.dma_start(out=ca_d[:, :], in_=out_sb[:, 0:128])
            nc.sync.dma_start(out=cx_d[:, :], in_=out_sb[:, 128:256])

    nc.compile()
    return nc


def _run(cfg, nc, planes, W_pool, W_embed, trace=False):
    c = cfg
    rmask = np.zeros((128, 1), np.float32)
    lr = c.PN - (c.NBLK - 1) * 128 if c.PN % 128 else 128
    rmask[:lr] = 1.0
    wp = np.ascontiguousarray(np.asarray(W_pool, np.float32)).astype(BF16)
    we = np.ascontiguousarray(np.asarray(W_embed, np.float32)).astype(BF16)
    in_maps = []
    for m in range(c.CORES):
        in_maps.append({
            "rmask": rmask, "wp": wp, "we": we,
            "gx": planes[m]["gx"], "mv": planes[m]["mv"],
            "mv2": planes[m]["mv2"], "idx2": planes[m]["idx2"],
            "ab": planes[m]["ab"],
        })
    res = run_bass_kernel_spmd(nc, in_maps, list(range(c.CORES)), trace=trace)
    ca = np.asarray(res.results[0]["coarse_A"], np.float32)
    cx = np.asarray(res.results[0]["coarse_X"], np.float32)
    return ca, cx, res


FULL = Cfg(n_nodes=50000, n_edges=1600000, cores=8, f_in=128, k_clust=128,
           grp_blocks=2, cstar_blocks=220)


def kernel(x, edge_row, edge_col, edge_val, W_pool, W_embed):
    _install_profile_hook()
    x = np.asarray(x, np.float32)
    edge_row = np.asarray(edge_row, np.int32)
    edge_col = np.asarray(edge_col, np.int32)
    edge_val = np.asarray(edge_val, np.float32)

    tiles1, tiles2, planes = _prep(FULL, x, edge_row, edge_col, edge_val)
    nc = _build(FULL, tiles1, tiles2)
    ca, cx, _ = _run(FULL, nc, planes, W_pool, W_embed)
    return ca, cx
